# revision 15
# baseline (speedup 1.0000x reference)
"""Trainium2 Bass kernel for the CNODE (HiPPO continuous-time ODE) model.

Strategy (8 NeuronCores, pure data parallel over batch B=256 -> 32/core):
  - All state kept feature-major in SBUF: partition = feature (mod 128),
    free dim = (feature_block, batch).
  - MLP layers are weight-stationary matmuls: lhsT = weight tile [K,128],
    rhs = activations [K, 32].  No transposes anywhere.
  - cn0 (HiPPO coefficients) update: one block-diagonal matmul (kron(I4, A'^T))
    over all 4 feature blocks at once (N=128) plus 4 tiny C-matmuls injecting
    the Bv * u forcing term.  DT is folded into A/Bv/W3/b3 on the host.
  - cn1 state feature order is permuted to [psi(512); y(16)] so the obs-step
    reset cn1 <- [y_t, cn0] is partition-aligned with cn0 (pure elementwise).
  - Sequential time loop (T=50 obs x 5 euler) fully unrolled.
"""

import os
from contextlib import ExitStack

import numpy as np
import ml_dtypes

Nc, ID, HID = 32, 16, 512
DT, N_STEPS, B, T = 0.05, 5, 256, 50
D0 = ID * Nc            # 512
DIN = D0 + ID           # 528
NCORES = 8
BL = B // NCORES        # 32 batch per core
DELTA = 5.0

USE_BF16 = os.environ.get("CNODE_BF16", "0") == "1"
USE_TELE = os.environ.get("CNODE_TELE", "1") == "1"
T_RUN = int(os.environ.get("CNODE_T_RUN", str(T)))  # dev knob; harness uses 50

_CACHE: dict = {}
LAST_RESULT = None


# ---------------------------------------------------------------- program ---
def _build(bf16: bool, t_run: int, tele: bool):
    import concourse.bass as bass
    from concourse import bacc, mybir, tile

    f32 = mybir.dt.float32
    dtw = mybir.dt.bfloat16 if bf16 else f32
    ADD = mybir.AluOpType.add
    SUB = mybir.AluOpType.subtract
    MUL = mybir.AluOpType.mult
    MAX = mybir.AluOpType.max

    nc = bacc.Bacc("TRN2", target_bir_lowering=False, debug=False,
                   num_devices=NCORES)

    w1 = nc.dram_tensor("w1", [128, 5 * 512], dtw, kind="ExternalInput").ap()
    w2 = nc.dram_tensor("w2", [128, 4 * 512], dtw, kind="ExternalInput").ap()
    w3 = nc.dram_tensor("w3", [128, 4 * 528], dtw, kind="ExternalInput").ap()
    bd = nc.dram_tensor("bd", [128, 128], f32, kind="ExternalInput").ap()
    cm = nc.dram_tensor("cm", [16, 512], f32, kind="ExternalInput").ap()
    if tele:
        w31 = nc.dram_tensor("w31", [128, 4 * 512], dtw,
                             kind="ExternalInput").ap()
    b1 = nc.dram_tensor("b1", [128, 4 * N_STEPS], f32,
                        kind="ExternalInput").ap()
    b2 = nc.dram_tensor("b2", [128, 4], f32, kind="ExternalInput").ap()
    b3 = nc.dram_tensor("b3", [128, 5], f32, kind="ExternalInput").ap()
    yt = nc.dram_tensor("yt", [16, 32 * t_run], f32, kind="ExternalInput").ap()
    mt = nc.dram_tensor("mt", [128, 128 * t_run], f32, kind="ExternalInput").ap()

    preds = nc.dram_tensor("preds", [16, 32 * t_run], f32, kind="ExternalOutput").ap()
    traj0 = nc.dram_tensor("traj0", [t_run, 128, 128], f32, kind="ExternalOutput").ap()
    fin1 = nc.dram_tensor("fin1", [128, 160], f32, kind="ExternalOutput").ap()

    with tile.TileContext(nc) as tc, ExitStack() as ctx:
        wp = ctx.enter_context(tc.tile_pool(name="weights", bufs=1))
        sp = ctx.enter_context(tc.tile_pool(name="state", bufs=1))
        ap_ = ctx.enter_context(tc.tile_pool(name="acts", bufs=2))
        pp = ctx.enter_context(tc.tile_pool(name="psum", bufs=2, space="PSUM"))

        W1t = wp.tile([128, 5 * 512], dtw)
        W2t = wp.tile([128, 4 * 512], dtw)
        W3t = wp.tile([128, 4 * 528], dtw)
        if tele:
            W31t = wp.tile([128, 4 * 512], dtw)
        BDt = wp.tile([128, 128], f32)
        Ct = wp.tile([16, 512], f32)
        b1t = wp.tile([128, 4 * N_STEPS], f32)
        b2t = wp.tile([128, 4], f32)
        b3t = wp.tile([128, 5], f32)
        Yt = wp.tile([16, 32 * t_run], f32)
        Mt = wp.tile([128, 128 * t_run], f32)
        predt = wp.tile([16, 32 * t_run], f32)

        dma_pairs = [(W1t, w1), (W2t, w2), (W3t, w3), (BDt, bd), (Ct, cm),
                     (b1t, b1), (b2t, b2), (b3t, b3), (Yt, yt), (Mt, mt)]
        if tele:
            dma_pairs.append((W31t, w31))
        for dst, src in dma_pairs:
            nc.sync.dma_start(dst[:], src[:])

        cn0 = sp.tile([128, 128], f32)
        cn1 = sp.tile([128, 160], f32)
        nc.vector.memset(cn0[:], 0.0)
        nc.vector.memset(cn1[:], 0.0)
        if bf16:
            cn1x = sp.tile([128, 160], dtw)
            nc.vector.memset(cn1x[:], 0.0)
        else:
            cn1x = cn1

        def x_slice(k):
            # MLP input K-tile k (feature-major): psi blocks 0..3, then y.
            if k < 4:
                return cn1x[:, 32 * k:32 * k + 32]
            return cn1x[0:16, 128:160]

        def euler_step(p1, s, h2_prev):
            p2 = pp.tile([128, 128], f32, tag="p2")
            p3 = pp.tile([128, 160], f32, tag="p3")
            pc = pp.tile([128, 128], f32, tag="pc")
            h1 = ap_.tile([128, 128], dtw, tag="h1")
            h2 = ap_.tile([128, 128], dtw, tag="h2")

            # L1: h1 = relu(L1psum + b1 + s*c0)
            # s==0: L1psum = W1p^T x (fresh, 528-contraction)
            # s>0 (tele): L1psum += W31^T h2_prev  (accumulate in PSUM)
            if tele and s > 0:
                for m in range(4):
                    for k in range(4):
                        nc.tensor.matmul(
                            p1[:, 32 * m:32 * m + 32],
                            W31t[:, k * 512 + m * 128:k * 512 + m * 128 + 128],
                            h2_prev[:, 32 * k:32 * k + 32],
                            start=False, stop=(k == 3), skip_group_check=True)
            else:
                for m in range(4):
                    for k in range(5):
                        kk = 128 if k < 4 else 16
                        # start=True clears has_written for the WHOLE bank, so
                        # in tele mode only the first MM of the interval may
                        # set it (later steps accumulate onto every m-tile).
                        st = (m == 0 and k == 0) if tele else (k == 0)
                        nc.tensor.matmul(
                            p1[:, 32 * m:32 * m + 32],
                            W1t[0:kk, k * 512 + m * 128:k * 512 + m * 128 + 128],
                            x_slice(k), start=st, stop=(k == 4),
                            skip_group_check=tele)
            bs = (s * 4) if tele else 0
            for m in range(4):
                nc.vector.tensor_scalar(
                    h1[:, 32 * m:32 * m + 32], p1[:, 32 * m:32 * m + 32],
                    b1t[:, bs + m:bs + m + 1], 0.0, ADD, MAX)

            # cn0 ODE: pc = BD^T cn0 (all 4 blocks, N=128) + C_j^T u
            nc.tensor.matmul(pc[:, 0:128], BDt[:, :], cn0[:, 0:128],
                             start=True, stop=False)
            for j in range(4):
                nc.tensor.matmul(pc[:, 32 * j:32 * j + 32],
                                 Ct[0:16, 128 * j:128 * j + 128],
                                 cn1[0:16, 128:160],
                                 start=False, stop=(j == 3),
                                 skip_group_check=True)

            # L2: h2 = relu(W2^T h1 + b2)
            for m in range(4):
                for k in range(4):
                    nc.tensor.matmul(
                        p2[:, 32 * m:32 * m + 32],
                        W2t[:, k * 512 + m * 128:k * 512 + m * 128 + 128],
                        h1[:, 32 * k:32 * k + 32], start=(k == 0), stop=(k == 3))
                nc.vector.tensor_scalar(
                    h2[:, 32 * m:32 * m + 32], p2[:, 32 * m:32 * m + 32],
                    b2t[:, m:m + 1], 0.0, ADD, MAX)

            # L3: p3 = W3p'^T h2   (DT and output permutation folded in)
            for m in range(5):
                mm = 128 if m < 4 else 16
                oc = (slice(0, 128), slice(32 * m, 32 * m + 32)) if m < 4 \
                    else (slice(0, 16), slice(128, 160))
                for k in range(4):
                    nc.tensor.matmul(
                        p3[oc[0], oc[1]],
                        W3t[:, k * 528 + m * 128:k * 528 + m * 128 + mm],
                        h2[:, 32 * k:32 * k + 32], start=(k == 0), stop=(k == 3))

            # cn1 += p3 + b3'   (bf16 shadow first: it gates next step's L1)
            for m in range(5):
                if m < 4:
                    po, co = slice(0, 128), slice(32 * m, 32 * m + 32)
                    bcol = b3t[:, m:m + 1]
                else:
                    po, co = slice(0, 16), slice(128, 160)
                    bcol = b3t[0:16, 4:5]
                if bf16 and not tele:
                    nc.vector.scalar_tensor_tensor(
                        cn1x[po, co], p3[po, co], bcol, cn1[po, co], ADD, ADD)
                nc.vector.scalar_tensor_tensor(
                    cn1[po, co], p3[po, co], bcol, cn1[po, co], ADD, ADD)

            # cn0 += pc
            nc.vector.tensor_tensor(cn0[:, :], pc[:, 0:128], cn0[:, :], ADD)
            return h2

        def obs_update(t):
            # prediction = y-part of h_end (pre-mix)
            nc.vector.tensor_copy(predt[:, 32 * t:32 * t + 32],
                                  cn1[0:16, 128:160])
            # cn1 <- cn1 + m * (upd - cn1); upd = [cn0 (psi); y_t]
            t1 = ap_.tile([128, 128], f32, tag="mix1")
            t2 = ap_.tile([128, 128], f32, tag="mix2")
            nc.vector.tensor_tensor(t1[:, :], cn0[:, :], cn1[:, 0:128], SUB)
            nc.vector.tensor_tensor(t2[:, :], t1[:, :],
                                    Mt[:, 128 * t:128 * t + 128], MUL)
            if bf16:
                nc.vector.tensor_tensor(cn1x[:, 0:128], t2[:, :],
                                        cn1[:, 0:128], ADD)
            nc.vector.tensor_tensor(cn1[:, 0:128], t2[:, :], cn1[:, 0:128], ADD)

            t1y = ap_.tile([16, 32], f32, tag="mix1y")
            t2y = ap_.tile([16, 32], f32, tag="mix2y")
            nc.vector.tensor_tensor(t1y[:, :], Yt[0:16, 32 * t:32 * t + 32],
                                    cn1[0:16, 128:160], SUB)
            nc.vector.tensor_tensor(t2y[:, :], t1y[:, :],
                                    Mt[0:16, 128 * t:128 * t + 32], MUL)
            if bf16:
                nc.vector.tensor_tensor(cn1x[0:16, 128:160], t2y[:, :],
                                        cn1[0:16, 128:160], ADD)
            nc.vector.tensor_tensor(cn1[0:16, 128:160], t2y[:, :],
                                    cn1[0:16, 128:160], ADD)

            nc.sync.dma_start(traj0[t], cn0[:, :])
            if t == t_run - 1:
                nc.sync.dma_start(fin1[:], cn1[:, :])

        for t in range(t_run):
            p1 = pp.tile([128, 128], f32, tag="p1")
            h2_prev = None
            for s in range(N_STEPS):
                if not tele and s > 0:
                    p1 = pp.tile([128, 128], f32, tag="p1")
                h2_prev = euler_step(p1, s, h2_prev)
            obs_update(t)

        nc.sync.dma_start(preds[:], predt[:])

    nc.compile()
    return nc


# ------------------------------------------------------------- host packing ---
def _pack_shared(inputs, bf16: bool, tele: bool):
    f32 = np.float32
    wdt = ml_dtypes.bfloat16 if bf16 else f32
    W1 = np.asarray(inputs["W1"], f32)
    W2 = np.asarray(inputs["W2"], f32)
    W3 = np.asarray(inputs["W3"], f32)
    b1v = np.asarray(inputs["b1"], f32)
    b2v = np.asarray(inputs["b2"], f32)
    b3v = np.asarray(inputs["b3"], f32)
    A = np.asarray(inputs["A"], f32)
    Bv = np.asarray(inputs["Bv"], f32)

    perm = np.r_[ID:DIN, 0:ID]             # cn1 features -> [psi; y]
    W1p = W1[perm, :]                      # [528, 512]
    W3p = (DT * W3)[:, perm]               # [512, 528]
    b3p = (DT * b3v)[perm]

    w1sb = np.zeros((128, 5 * 512), f32)
    for k in range(5):
        kk = 128 if k < 4 else 16
        w1sb[0:kk, k * 512:(k + 1) * 512] = W1p[k * 128:k * 128 + kk, :]
    w2sb = np.zeros((128, 4 * 512), f32)
    for k in range(4):
        w2sb[:, k * 512:(k + 1) * 512] = W2[k * 128:(k + 1) * 128, :]
    w3sb = np.zeros((128, 4 * 528), f32)
    for k in range(4):
        w3sb[:, k * 528:(k + 1) * 528] = W3p[k * 128:(k + 1) * 128, :]

    bdsb = np.kron(np.eye(4, dtype=f32), (DT * A).T).astype(f32)   # [128,128]
    cmsb = np.zeros((16, 512), f32)
    for i in range(16):
        base = 128 * (i // 4) + 32 * (i % 4)
        cmsb[i, base:base + 32] = DT * Bv

    b2sb = b2v.reshape(4, 128).T.copy()
    b3sb = np.zeros((128, 5), f32)
    b3sb[:, 0:4] = b3p[0:512].reshape(4, 128).T
    b3sb[0:16, 4] = b3p[512:528]

    c0 = DT * (W1.T @ b3v)                 # telescoped bias increment [512]
    b1sb = np.zeros((128, 4 * N_STEPS), f32)
    for s in range(N_STEPS):
        b1sb[:, 4 * s:4 * s + 4] = (b1v + s * c0).reshape(4, 128).T

    out = {"w1": w1sb.astype(wdt), "w2": w2sb.astype(wdt),
           "w3": w3sb.astype(wdt), "bd": bdsb, "cm": cmsb,
           "b1": b1sb, "b2": b2sb, "b3": b3sb}
    if tele:
        W31 = (DT * (W3 @ W1)).astype(f32)  # [512, 512]
        w31sb = np.zeros((128, 4 * 512), f32)
        for k in range(4):
            w31sb[:, k * 512:(k + 1) * 512] = W31[k * 128:(k + 1) * 128, :]
        out["w31"] = w31sb.astype(wdt)
    return out


def _pack_core(inputs, c: int, t_run: int):
    f32 = np.float32
    Y = np.asarray(inputs["Y"], f32)[c * BL:(c + 1) * BL]       # [32, T, 16]
    mask = np.asarray(inputs["mask"], f32)[c * BL:(c + 1) * BL]  # [32, T]
    ysb = Y[:, 0:t_run, :].transpose(2, 1, 0).reshape(16, t_run * 32).copy()
    mtb = mask[:, 0:t_run].T                                    # [t, b]
    mrow = np.tile(mtb, (1, 4)).reshape(1, t_run * 128)
    mtsb = np.broadcast_to(mrow, (128, t_run * 128)).copy()
    return {"yt": ysb, "mt": mtsb}


def kernel(**inputs):
    from concourse.bass_utils import run_bass_kernel_spmd

    bf16, t_run, tele = USE_BF16, T_RUN, USE_TELE
    key = (bf16, t_run, tele)
    if key not in _CACHE:
        _CACHE[key] = _build(bf16, t_run, tele)
    nc = _CACHE[key]

    shared = _pack_shared(inputs, bf16, tele)
    in_maps = [dict(shared, **_pack_core(inputs, c, t_run))
               for c in range(NCORES)]

    trace = os.environ.get("CNODE_TRACE", "0") == "1"
    res = run_bass_kernel_spmd(nc, in_maps, core_ids=list(range(NCORES)),
                               trace=trace)
    global LAST_RESULT
    LAST_RESULT = res
    outs = res.results

    f32 = np.float32
    mask = np.asarray(inputs["mask"], f32)[:, 0:t_run]
    y_preds = np.zeros((B, t_run, ID), f32)
    h_fin = np.zeros((B, 2 * D0 + ID), f32)
    cn0_traj = np.zeros((t_run, B, D0), f32)
    for c in range(NCORES):
        o = outs[c]
        sl = slice(c * BL, (c + 1) * BL)
        y_preds[sl] = np.asarray(o["preds"], f32).reshape(
            16, t_run, 32).transpose(2, 1, 0)
        tr = np.asarray(o["traj0"], f32).reshape(
            t_run, 128, 4, 32).transpose(0, 3, 2, 1).reshape(t_run, 32, D0)
        cn0_traj[:, sl, :] = tr
        f1 = np.asarray(o["fin1"], f32)
        psi = f1[:, 0:128].reshape(128, 4, 32).transpose(2, 1, 0).reshape(32, D0)
        yfin = f1[0:16, 128:160].T
        h_fin[sl, 0:D0] = tr[t_run - 1]
        h_fin[sl, D0:D0 + ID] = yfin
        h_fin[sl, D0 + ID:] = psi

    any_t = np.nonzero(mask.any(axis=0))[0]
    if len(any_t):
        last_h_cn = cn0_traj[any_t.max()].copy()
    else:
        last_h_cn = np.zeros((B, D0), f32)

    times = np.asarray(inputs["times"], f32)
    return y_preds, y_preds.copy(), times, last_h_cn, h_fin


# revision 18
# speedup vs baseline: 556.7385x; 556.7385x over previous
"""Trainium2 Bass kernel for the CNODE (HiPPO continuous-time ODE) model.

Strategy (8 NeuronCores, pure data parallel over batch B=256 -> 32/core):
  - All state kept feature-major in SBUF: partition = feature (mod 128),
    free dim = (feature_block, batch).
  - MLP layers are weight-stationary matmuls: lhsT = weight tile [K,128],
    rhs = activations [K, 32].  No transposes anywhere.
  - cn0 (HiPPO coefficients) update: one block-diagonal matmul (kron(I4, A'^T))
    over all 4 feature blocks at once (N=128) plus 4 tiny C-matmuls injecting
    the Bv * u forcing term.  DT is folded into A/Bv/W3/b3 on the host.
  - cn1 state feature order is permuted to [psi(512); y(16)] so the obs-step
    reset cn1 <- [y_t, cn0] is partition-aligned with cn0 (pure elementwise).
  - Sequential time loop (T=50 obs x 5 euler) fully unrolled.
"""

import os
from contextlib import ExitStack

import numpy as np
import ml_dtypes

Nc, ID, HID = 32, 16, 512
DT, N_STEPS, B, T = 0.05, 5, 256, 50
D0 = ID * Nc            # 512
DIN = D0 + ID           # 528
NCORES = 8
BL = B // NCORES        # 32 batch per core
DELTA = 5.0

USE_BF16 = os.environ.get("CNODE_BF16", "0") == "1"
USE_TELE = os.environ.get("CNODE_TELE", "1") == "1"
T_RUN = int(os.environ.get("CNODE_T_RUN", str(T)))  # dev knob; harness uses 50

_CACHE: dict = {}
LAST_RESULT = None


# ---------------------------------------------------------------- program ---
def _build(bf16: bool, t_run: int, tele: bool, reps: int = 1):
    import concourse.bass as bass
    from concourse import bacc, mybir, tile

    f32 = mybir.dt.float32
    dtw = mybir.dt.bfloat16 if bf16 else f32
    ADD = mybir.AluOpType.add
    SUB = mybir.AluOpType.subtract
    MUL = mybir.AluOpType.mult
    MAX = mybir.AluOpType.max

    nc = bacc.Bacc("TRN2", target_bir_lowering=False, debug=False,
                   num_devices=NCORES)

    w1 = nc.dram_tensor("w1", [128, 5 * 512], dtw, kind="ExternalInput").ap()
    w2 = nc.dram_tensor("w2", [128, 4 * 512], dtw, kind="ExternalInput").ap()
    w3 = nc.dram_tensor("w3", [128, 4 * 528], dtw, kind="ExternalInput").ap()
    bd = nc.dram_tensor("bd", [128, 128], f32, kind="ExternalInput").ap()
    cm = nc.dram_tensor("cm", [16, 512], f32, kind="ExternalInput").ap()
    if tele:
        w31 = nc.dram_tensor("w31", [128, 4 * 512], dtw,
                             kind="ExternalInput").ap()
    b1 = nc.dram_tensor("b1", [128, 4 * N_STEPS], f32,
                        kind="ExternalInput").ap()
    b2 = nc.dram_tensor("b2", [128, 4], f32, kind="ExternalInput").ap()
    b3 = nc.dram_tensor("b3", [128, 5], f32, kind="ExternalInput").ap()
    yt = nc.dram_tensor("yt", [16, 32 * t_run], f32, kind="ExternalInput").ap()
    mt = nc.dram_tensor("mt", [128, 128 * t_run], f32, kind="ExternalInput").ap()

    preds = nc.dram_tensor("preds", [16, 32 * t_run], f32, kind="ExternalOutput").ap()
    traj0 = nc.dram_tensor("traj0", [t_run, 128, 128], f32, kind="ExternalOutput").ap()
    fin1 = nc.dram_tensor("fin1", [128, 160], f32, kind="ExternalOutput").ap()

    with tile.TileContext(nc) as tc, ExitStack() as ctx:
        wp = ctx.enter_context(tc.tile_pool(name="weights", bufs=1))
        sp = ctx.enter_context(tc.tile_pool(name="state", bufs=1))
        ap_ = ctx.enter_context(tc.tile_pool(name="acts", bufs=2))
        pp = ctx.enter_context(tc.tile_pool(name="psum", bufs=2, space="PSUM"))

        W1t = wp.tile([128, 5 * 512], dtw)
        W2t = wp.tile([128, 4 * 512], dtw)
        W3t = wp.tile([128, 4 * 528], dtw)
        if tele:
            W31t = wp.tile([128, 4 * 512], dtw)
        BDt = wp.tile([128, 128], f32)
        Ct = wp.tile([16, 512], f32)
        b1t = wp.tile([128, 4 * N_STEPS], f32)
        b2t = wp.tile([128, 4], f32)
        b3t = wp.tile([128, 5], f32)
        Yt = wp.tile([16, 32 * t_run], f32)
        Mt = wp.tile([128, 128 * t_run], f32)
        predt = wp.tile([16, 32 * t_run], f32)

        dma_pairs = [(W1t, w1), (W2t, w2), (W3t, w3), (BDt, bd), (Ct, cm),
                     (b1t, b1), (b2t, b2), (b3t, b3), (Yt, yt), (Mt, mt)]
        if tele:
            dma_pairs.append((W31t, w31))
        for dst, src in dma_pairs:
            nc.sync.dma_start(dst[:], src[:])

        cn0 = sp.tile([128, 128], f32)
        cn1 = sp.tile([128, 160], f32)
        if bf16:
            cn1x = sp.tile([128, 160], dtw)
        else:
            cn1x = cn1

        def x_slice(k):
            # MLP input K-tile k (feature-major): psi blocks 0..3, then y.
            if k < 4:
                return cn1x[:, 32 * k:32 * k + 32]
            return cn1x[0:16, 128:160]

        def euler_step(p1, s, h2_prev):
            p2 = pp.tile([128, 128], f32, tag="p2")
            p3 = pp.tile([128, 160], f32, tag="p3")
            pc = pp.tile([128, 128], f32, tag="pc")
            h1 = ap_.tile([128, 128], dtw, tag="h1")
            h2 = ap_.tile([128, 128], dtw, tag="h2")

            # L1: h1 = relu(L1psum + b1 + s*c0)
            # s==0: L1psum = W1p^T x (fresh, 528-contraction)
            # s>0 (tele): L1psum += W31^T h2_prev  (accumulate in PSUM)
            if tele and s > 0:
                for m in range(4):
                    for k in range(4):
                        nc.tensor.matmul(
                            p1[:, 32 * m:32 * m + 32],
                            W31t[:, k * 512 + m * 128:k * 512 + m * 128 + 128],
                            h2_prev[:, 32 * k:32 * k + 32],
                            start=False, stop=(k == 3), skip_group_check=True)
            else:
                for m in range(4):
                    for k in range(5):
                        kk = 128 if k < 4 else 16
                        # start=True clears has_written for the WHOLE bank, so
                        # in tele mode only the first MM of the interval may
                        # set it (later steps accumulate onto every m-tile).
                        st = (m == 0 and k == 0) if tele else (k == 0)
                        nc.tensor.matmul(
                            p1[:, 32 * m:32 * m + 32],
                            W1t[0:kk, k * 512 + m * 128:k * 512 + m * 128 + 128],
                            x_slice(k), start=st, stop=(k == 4),
                            skip_group_check=tele)
            bs = (s * 4) if tele else 0
            for m in range(4):
                nc.vector.tensor_scalar(
                    h1[:, 32 * m:32 * m + 32], p1[:, 32 * m:32 * m + 32],
                    b1t[:, bs + m:bs + m + 1], 0.0, ADD, MAX)

            # cn0 ODE: pc = BD^T cn0 (all 4 blocks, N=128) + C_j^T u
            nc.tensor.matmul(pc[:, 0:128], BDt[:, :], cn0[:, 0:128],
                             start=True, stop=False)
            for j in range(4):
                nc.tensor.matmul(pc[:, 32 * j:32 * j + 32],
                                 Ct[0:16, 128 * j:128 * j + 128],
                                 cn1[0:16, 128:160],
                                 start=False, stop=(j == 3),
                                 skip_group_check=True)

            # L2: h2 = relu(W2^T h1 + b2)
            for m in range(4):
                for k in range(4):
                    nc.tensor.matmul(
                        p2[:, 32 * m:32 * m + 32],
                        W2t[:, k * 512 + m * 128:k * 512 + m * 128 + 128],
                        h1[:, 32 * k:32 * k + 32], start=(k == 0), stop=(k == 3))
                nc.vector.tensor_scalar(
                    h2[:, 32 * m:32 * m + 32], p2[:, 32 * m:32 * m + 32],
                    b2t[:, m:m + 1], 0.0, ADD, MAX)

            # L3: p3 = W3p'^T h2   (DT and output permutation folded in)
            for m in range(5):
                mm = 128 if m < 4 else 16
                oc = (slice(0, 128), slice(32 * m, 32 * m + 32)) if m < 4 \
                    else (slice(0, 16), slice(128, 160))
                for k in range(4):
                    nc.tensor.matmul(
                        p3[oc[0], oc[1]],
                        W3t[:, k * 528 + m * 128:k * 528 + m * 128 + mm],
                        h2[:, 32 * k:32 * k + 32], start=(k == 0), stop=(k == 3))

            # cn1 += p3 + b3'   (bf16 shadow first: it gates next step's L1)
            for m in range(5):
                if m < 4:
                    po, co = slice(0, 128), slice(32 * m, 32 * m + 32)
                    bcol = b3t[:, m:m + 1]
                else:
                    po, co = slice(0, 16), slice(128, 160)
                    bcol = b3t[0:16, 4:5]
                if bf16 and not tele:
                    nc.vector.scalar_tensor_tensor(
                        cn1x[po, co], p3[po, co], bcol, cn1[po, co], ADD, ADD)
                nc.vector.scalar_tensor_tensor(
                    cn1[po, co], p3[po, co], bcol, cn1[po, co], ADD, ADD)

            # cn0 += pc
            nc.vector.tensor_tensor(cn0[:, :], pc[:, 0:128], cn0[:, :], ADD)
            return h2

        def obs_update(t):
            # prediction = y-part of h_end (pre-mix)
            nc.vector.tensor_copy(predt[:, 32 * t:32 * t + 32],
                                  cn1[0:16, 128:160])
            # cn1 <- cn1 + m * (upd - cn1); upd = [cn0 (psi); y_t]
            t1 = ap_.tile([128, 128], f32, tag="mix1")
            t2 = ap_.tile([128, 128], f32, tag="mix2")
            nc.vector.tensor_tensor(t1[:, :], cn0[:, :], cn1[:, 0:128], SUB)
            nc.vector.tensor_tensor(t2[:, :], t1[:, :],
                                    Mt[:, 128 * t:128 * t + 128], MUL)
            if bf16:
                nc.vector.tensor_tensor(cn1x[:, 0:128], t2[:, :],
                                        cn1[:, 0:128], ADD)
            nc.vector.tensor_tensor(cn1[:, 0:128], t2[:, :], cn1[:, 0:128], ADD)

            t1y = ap_.tile([16, 32], f32, tag="mix1y")
            t2y = ap_.tile([16, 32], f32, tag="mix2y")
            nc.vector.tensor_tensor(t1y[:, :], Yt[0:16, 32 * t:32 * t + 32],
                                    cn1[0:16, 128:160], SUB)
            nc.vector.tensor_tensor(t2y[:, :], t1y[:, :],
                                    Mt[0:16, 128 * t:128 * t + 32], MUL)
            if bf16:
                nc.vector.tensor_tensor(cn1x[0:16, 128:160], t2y[:, :],
                                        cn1[0:16, 128:160], ADD)
            nc.vector.tensor_tensor(cn1[0:16, 128:160], t2y[:, :],
                                    cn1[0:16, 128:160], ADD)

            nc.sync.dma_start(traj0[t], cn0[:, :])
            if t == t_run - 1:
                nc.sync.dma_start(fin1[:], cn1[:, :])

        for _rep in range(reps):
            nc.vector.memset(cn0[:], 0.0)
            nc.vector.memset(cn1[:], 0.0)
            if bf16:
                nc.vector.memset(cn1x[:], 0.0)
            for t in range(t_run):
                p1 = pp.tile([128, 128], f32, tag="p1")
                h2_prev = None
                for s in range(N_STEPS):
                    if not tele and s > 0:
                        p1 = pp.tile([128, 128], f32, tag="p1")
                    h2_prev = euler_step(p1, s, h2_prev)
                obs_update(t)

        nc.sync.dma_start(preds[:], predt[:])

    nc.compile()
    return nc


# ------------------------------------------------------------- host packing ---
def _pack_shared(inputs, bf16: bool, tele: bool):
    f32 = np.float32
    wdt = ml_dtypes.bfloat16 if bf16 else f32
    W1 = np.asarray(inputs["W1"], f32)
    W2 = np.asarray(inputs["W2"], f32)
    W3 = np.asarray(inputs["W3"], f32)
    b1v = np.asarray(inputs["b1"], f32)
    b2v = np.asarray(inputs["b2"], f32)
    b3v = np.asarray(inputs["b3"], f32)
    A = np.asarray(inputs["A"], f32)
    Bv = np.asarray(inputs["Bv"], f32)

    perm = np.r_[ID:DIN, 0:ID]             # cn1 features -> [psi; y]
    W1p = W1[perm, :]                      # [528, 512]
    W3p = (DT * W3)[:, perm]               # [512, 528]
    b3p = (DT * b3v)[perm]

    w1sb = np.zeros((128, 5 * 512), f32)
    for k in range(5):
        kk = 128 if k < 4 else 16
        w1sb[0:kk, k * 512:(k + 1) * 512] = W1p[k * 128:k * 128 + kk, :]
    w2sb = np.zeros((128, 4 * 512), f32)
    for k in range(4):
        w2sb[:, k * 512:(k + 1) * 512] = W2[k * 128:(k + 1) * 128, :]
    w3sb = np.zeros((128, 4 * 528), f32)
    for k in range(4):
        w3sb[:, k * 528:(k + 1) * 528] = W3p[k * 128:(k + 1) * 128, :]

    bdsb = np.kron(np.eye(4, dtype=f32), (DT * A).T).astype(f32)   # [128,128]
    cmsb = np.zeros((16, 512), f32)
    for i in range(16):
        base = 128 * (i // 4) + 32 * (i % 4)
        cmsb[i, base:base + 32] = DT * Bv

    b2sb = b2v.reshape(4, 128).T.copy()
    b3sb = np.zeros((128, 5), f32)
    b3sb[:, 0:4] = b3p[0:512].reshape(4, 128).T
    b3sb[0:16, 4] = b3p[512:528]

    c0 = DT * (W1.T @ b3v)                 # telescoped bias increment [512]
    b1sb = np.zeros((128, 4 * N_STEPS), f32)
    for s in range(N_STEPS):
        b1sb[:, 4 * s:4 * s + 4] = (b1v + s * c0).reshape(4, 128).T

    out = {"w1": w1sb.astype(wdt), "w2": w2sb.astype(wdt),
           "w3": w3sb.astype(wdt), "bd": bdsb, "cm": cmsb,
           "b1": b1sb, "b2": b2sb, "b3": b3sb}
    if tele:
        W31 = (DT * (W3 @ W1)).astype(f32)  # [512, 512]
        w31sb = np.zeros((128, 4 * 512), f32)
        for k in range(4):
            w31sb[:, k * 512:(k + 1) * 512] = W31[k * 128:(k + 1) * 128, :]
        out["w31"] = w31sb.astype(wdt)
    return out


def _pack_core(inputs, c: int, t_run: int):
    f32 = np.float32
    Y = np.asarray(inputs["Y"], f32)[c * BL:(c + 1) * BL]       # [32, T, 16]
    mask = np.asarray(inputs["mask"], f32)[c * BL:(c + 1) * BL]  # [32, T]
    ysb = Y[:, 0:t_run, :].transpose(2, 1, 0).reshape(16, t_run * 32).copy()
    mtb = mask[:, 0:t_run].T                                    # [t, b]
    mrow = np.tile(mtb, (1, 4)).reshape(1, t_run * 128)
    mtsb = np.broadcast_to(mrow, (128, t_run * 128)).copy()
    return {"yt": ysb, "mt": mtsb}


def kernel(**inputs):
    from concourse.bass_utils import run_bass_kernel_spmd

    bf16, t_run, tele = USE_BF16, T_RUN, USE_TELE
    key = (bf16, t_run, tele)
    if key not in _CACHE:
        _CACHE[key] = _build(bf16, t_run, tele)
    nc = _CACHE[key]

    shared = _pack_shared(inputs, bf16, tele)
    in_maps = [dict(shared, **_pack_core(inputs, c, t_run))
               for c in range(NCORES)]

    trace = os.environ.get("CNODE_TRACE", "0") == "1"
    res = run_bass_kernel_spmd(nc, in_maps, core_ids=list(range(NCORES)),
                               trace=trace)
    global LAST_RESULT
    LAST_RESULT = res
    outs = res.results

    f32 = np.float32
    mask = np.asarray(inputs["mask"], f32)[:, 0:t_run]
    y_preds = np.zeros((B, t_run, ID), f32)
    h_fin = np.zeros((B, 2 * D0 + ID), f32)
    cn0_traj = np.zeros((t_run, B, D0), f32)
    for c in range(NCORES):
        o = outs[c]
        sl = slice(c * BL, (c + 1) * BL)
        y_preds[sl] = np.asarray(o["preds"], f32).reshape(
            16, t_run, 32).transpose(2, 1, 0)
        tr = np.asarray(o["traj0"], f32).reshape(
            t_run, 128, 4, 32).transpose(0, 3, 2, 1).reshape(t_run, 32, D0)
        cn0_traj[:, sl, :] = tr
        f1 = np.asarray(o["fin1"], f32)
        psi = f1[:, 0:128].reshape(128, 4, 32).transpose(2, 1, 0).reshape(32, D0)
        yfin = f1[0:16, 128:160].T
        h_fin[sl, 0:D0] = tr[t_run - 1]
        h_fin[sl, D0:D0 + ID] = yfin
        h_fin[sl, D0 + ID:] = psi

    any_t = np.nonzero(mask.any(axis=0))[0]
    if len(any_t):
        last_h_cn = cn0_traj[any_t.max()].copy()
    else:
        last_h_cn = np.zeros((B, D0), f32)

    times = np.asarray(inputs["times"], f32)
    return y_preds, y_preds.copy(), times, last_h_cn, h_fin
